# revision 57
# baseline (speedup 1.0000x reference)
"""Sequence-parallel DeepSpeed encoder-decoder block on 8 TRN2 NeuronCores.

Each core owns a 512-token block (cores 0-3 -> sequence 0, cores 4-7 ->
sequence 1) and computes the FULL layer for its tokens: LN1 + QKV (all 16
heads), attention against the whole sequence, attn_ow, residual, LN2, and
the complete MLP. The only communication is an AllGather of k and v^T
within each sequence's 4-core group; the final output is token-sharded so
the host just concatenates - no ReduceScatter at all.

v2 changes vs the first working kernel:
- A tiny warm-up AllGather issued at t=0 absorbs the ~40us collective
  rendezvous behind the P1 GEMMs instead of stalling attention.
- V is produced directly in transposed layout (DoubleRow GEMM with the
  x tiles stationary), AllGathered as [tokens, feats]; the per-pair PE
  transposes + PSUM drains of v1 are gone.  The augmented ones columns
  (softmax denominator) persist in SBUF; DMA fills only the v slices.
- exp() is written as fp8e4 in a DoubleRow-paired layout, so the ctx
  matmuls run DoubleRow (half the instructions of v1).
- LayerNorm rstd is folded INTO the fp8 activation tiles (x3n/res3n),
  so every QKV/MLP PSUM drain is a plain tensor_scalar instead of a
  tensor_tensor against the rstd broadcast; 1/std comes from exp(-.5 ln)
  on two [1,512] rows (same ACT table set as attention) instead of the
  slow DVE reciprocal on [128,512].
"""
from contextlib import ExitStack

import numpy as np
import ml_dtypes

import concourse.bacc as bacc
import concourse.mybir as mybir
import concourse.tile as tile
from concourse.bass_utils import run_bass_kernel_spmd

f32 = mybir.dt.float32
bf16 = mybir.dt.bfloat16
f8 = mybir.dt.float8e4
AF = mybir.ActivationFunctionType
ALU = mybir.AluOpType

NC = 8
B, S, D, I = 2, 2048, 1024, 4096
H, HD = 16, 64
T = B * S
TOK = T // NC        # 512 tokens per core
DC = D // 128        # 8 feature chunks
IC = I // 128        # 32 intermediate chunks
NKC = S // 128       # 16 key chunks per sequence
NJJ = NKC // 2       # 8 key-pair chunks (DoubleRow ctx granularity)
NR = 4               # ranks per sequence group
EPS = 1e-12

_BF = ml_dtypes.bfloat16


def _bf16(a):
    return np.ascontiguousarray(a.astype(_BF))


_F8 = ml_dtypes.float8_e4m3


def _f8(a):
    return np.ascontiguousarray(a.astype(_F8))


def _build():
    nc = bacc.Bacc("TRN2", target_bir_lowering=False, debug=False, num_devices=NC)

    inp = {}
    def din(name, shape, dt):
        inp[name] = nc.dram_tensor(name, shape, dt, kind="ExternalInput")
        return inp[name]

    xbf = din("xbf", [D, TOK], bf16)
    wqblk = din("wqblk", [3, DC, 128, DC, 128], f8)
    wvr = din("wvr", [DC // 2, 128, 2, D], f8)
    owblk = din("owblk", [DC, 128, DC, 128], f8)
    w1blk = din("w1blk", [IC, 128, DC, 128], f8)
    w2blk = din("w2blk", [IC, 128, DC, 128], f8)
    outwblk = din("outwblk", [DC, 128, IC, 128], f8)

    outT = nc.dram_tensor("outT", [D, TOK], f32, kind="ExternalOutput")

    RG2 = [[0, 1, 2, 3], [4, 5, 6, 7]]

    with tile.TileContext(nc) as tc:
        with ExitStack() as ctx:
            ep = ctx.enter_context
            # ---- global pools (whole-kernel lifetime) ----
            cons = ep(tc.tile_pool(name="cons", bufs=1))
            qp = ep(tc.tile_pool(name="qp", bufs=1))
            ctxp = ep(tc.tile_pool(name="ctxp", bufs=1))
            rowp = ep(tc.tile_pool(name="rowp", bufs=10))
            xresp = ep(tc.tile_pool(name="xresp", bufs=1))
            rstdp = ep(tc.tile_pool(name="rstdp", bufs=2))
            wfp = ep(tc.tile_pool(name="wfp", bufs=6))
            psp = ep(tc.tile_pool(name="psp", bufs=4, space="PSUM"))
            psB = ep(tc.tile_pool(name="psB", bufs=2, space="PSUM"))
            dram = ep(tc.tile_pool(name="dram", bufs=1, space="DRAM"))

            # DRAM scratch for the kv AllGathers, split into feature halves
            # so attention on head-pairs 0-3 can start after the first
            # quarter of the (serialized, BW-bound) collective chain.
            HD2 = D // 2
            k_own = [dram.tile([HD2, TOK], f8, tag=f"k_own{i}", name=f"k_own{i}")
                     for i in range(2)]
            v_ownT = [dram.tile([TOK, HD2], f8, tag=f"v_ownT{i}",
                                name=f"v_ownT{i}") for i in range(2)]
            k_ag = [dram.tile([NR * HD2, TOK], f8, tag=f"k_ag{i}",
                              name=f"k_ag{i}") for i in range(2)]
            v_agT = [dram.tile([NR * TOK, HD2], f8, tag=f"v_agT{i}",
                               name=f"v_agT{i}") for i in range(2)]

            # ---- constants ----
            ones_col = cons.tile([128, 1], bf16)
            nc.gpsimd.memset(ones_col[:], 1.0)
            ones_all = cons.tile([128, 64], bf16)
            nc.gpsimd.memset(ones_all[:], 1.0)
            ones_row128 = cons.tile([1, 128], bf16)
            nc.gpsimd.memset(ones_row128[:], 1.0)
            invD_f = cons.tile([1, 128], f32)
            nc.gpsimd.memset(invD_f[:], 1.0 / D)
            invD_row = cons.tile([1, 128], bf16)
            nc.vector.tensor_copy(invD_row[:], invD_f[:])
            eps_one = cons.tile([1, 1], f32)
            nc.gpsimd.memset(eps_one[:], EPS)



            RMAGIC = 0x5F3759DF  # fp32 rsqrt bit-trick constant

            def ln_stats(feed_tile_fn):
                """feed_tile_fn(d) -> bf16 [128,TOK] AP.  Returns
                (rstd_bc [128,TOK] f32 SBUF broadcast, mt_row [1,TOK] bf16
                = mean*rstd).  1/sqrt(var) runs entirely on DVE (bit-trick
                + one Newton step) so ACT's exp table set stays resident."""
                sum_ps = psp.tile([1, TOK], f32, tag="ps")
                ssq_ps = psp.tile([1, TOK], f32, tag="ps")
                for d in range(DC):
                    xt = feed_tile_fn(d)
                    sq = wfp.tile([128, TOK], bf16, tag="wf")
                    nc.vector.tensor_tensor(sq[:], xt, xt, op=ALU.mult)
                    nc.tensor.matmul(sum_ps[:], ones_col[:], xt,
                                     start=(d == 0), stop=(d == DC - 1))
                    nc.tensor.matmul(ssq_ps[:], ones_col[:], sq[:],
                                     start=(d == 0), stop=(d == DC - 1))
                mean_r = rowp.tile([1, TOK], f32, tag="rowf")
                nc.vector.tensor_scalar(mean_r[:], sum_ps[:], 1.0 / D, None,
                                        op0=ALU.mult)
                msq_r = rowp.tile([1, TOK], f32, tag="rowf")
                nc.vector.tensor_tensor(msq_r[:], mean_r[:], mean_r[:],
                                        op=ALU.mult)
                var_r = rowp.tile([1, TOK], f32, tag="rowf")
                nc.vector.scalar_tensor_tensor(var_r[:], ssq_ps[:], 1.0 / D,
                                               msq_r[:], op0=ALU.mult,
                                               op1=ALU.subtract)
                # rstd = rsqrt(var): x0 = bitcast(RMAGIC - (bits >> 1));
                # rstd = x0 * (1.5 - 0.5 * var * x0^2)
                sh_r = rowp.tile([1, TOK], f32, tag="rowf")
                nc.vector.tensor_scalar(sh_r[:].bitcast(mybir.dt.int32),
                                        var_r[:].bitcast(mybir.dt.int32),
                                        1, None, op0=ALU.arith_shift_right)
                x0_r = rowp.tile([1, TOK], f32, tag="rowf")
                nc.vector.tensor_scalar(x0_r[:].bitcast(mybir.dt.int32),
                                        sh_r[:].bitcast(mybir.dt.int32),
                                        -1, RMAGIC, op0=ALU.mult, op1=ALU.add)
                s1_r = rowp.tile([1, TOK], f32, tag="rowf")
                nc.vector.tensor_tensor(s1_r[:], x0_r[:], x0_r[:], op=ALU.mult)
                s2_r = rowp.tile([1, TOK], f32, tag="rowf")
                nc.vector.tensor_tensor(s2_r[:], var_r[:], s1_r[:], op=ALU.mult)
                w_r = rowp.tile([1, TOK], f32, tag="rowf")
                nc.vector.tensor_scalar(w_r[:], s2_r[:], -0.5, 1.5,
                                        op0=ALU.mult, op1=ALU.add)
                rstd_r = rowp.tile([1, TOK], bf16, tag="row")
                nc.vector.tensor_tensor(rstd_r[:], w_r[:], x0_r[:], op=ALU.mult)
                # broadcast 1/std and mean to all 128 partitions via PE
                bc_ps = psp.tile([128, TOK], f32, tag="ps")
                nc.tensor.matmul(bc_ps[:], ones_row128[:], rstd_r[:],
                                 start=True, stop=True)
                rstd_bc = rstdp.tile([128, TOK], f32, tag="rstd")
                nc.vector.tensor_copy(rstd_bc[:], bc_ps[:])
                mean_b = rowp.tile([1, TOK], bf16, tag="row")
                nc.vector.tensor_copy(mean_b[:], mean_r[:])
                mb_ps = psp.tile([128, TOK], f32, tag="ps")
                nc.tensor.matmul(mb_ps[:], ones_row128[:], mean_b[:],
                                 start=True, stop=True)
                mean_bc = rstdp.tile([128, TOK], f32, tag="mbc")
                nc.vector.tensor_copy(mean_bc[:], mb_ps[:])
                return rstd_bc, mean_bc

            # ================= P1: LN1 + QKV (k, vT, then q) =================
            p1s = ExitStack()
            wqp = p1s.enter_context(tc.tile_pool(name="wqp", bufs=6))
            xbp = p1s.enter_context(tc.tile_pool(name="xbp", bufs=1))
            kvoutp = p1s.enter_context(tc.tile_pool(name="kvoutp", bufs=4))
            wvp = p1s.enter_context(tc.tile_pool(name="wvp", bufs=4))
            vtp = p1s.enter_context(tc.tile_pool(name="vtp", bufs=2))

            xb = []
            for d in range(DC):
                t = xresp.tile([128, TOK], bf16, tag=f"xb{d}")
                nc.sync.dma_start(t[:], xbf[128 * d:128 * (d + 1), :])
                xb.append(t)

            rstd1, mean1 = ln_stats(lambda d: xb[d][:])

            # fully normalized fp8 x tiles ((x-mean)*rstd), DoubleRow pairs:
            # no rank-1 mean-correction matmuls needed anywhere downstream
            x3n = [xbp.tile([128, 2, TOK], f8, tag=f"x3n{c}", name=f"x3n{c}")
                   for c in range(DC // 2)]
            for d in range(DC):
                eng = nc.vector if d % 2 == 0 else nc.gpsimd
                xc = wfp.tile([128, TOK], bf16, tag="wf")
                eng.tensor_tensor(xc[:], xb[d][:], mean1[:],
                                  op=ALU.subtract)
                eng.tensor_tensor(x3n[d // 2][:, d % 2, :], xc[:],
                                  rstd1[:], op=ALU.mult)

            DR = mybir.MatmulPerfMode.DoubleRow

            # ---- k then q GEMMs (fp8 DR), vT GEMM in between ----
            def qk_gemm(part, out_cb):
                for j in range(DC):
                    wqt = wqp.tile([128, DC, 128], f8, tag="wqs")
                    nc.sync.dma_start(wqt[:], wqblk[part, j])
                    ps = psp.tile([128, TOK], f32, tag="ps")
                    for c in range(DC // 2):
                        nc.tensor.matmul(ps[:], wqt[:, 2 * c:2 * c + 2, :],
                                         x3n[c][:], perf_mode=DR,
                                         start=(c == 0),
                                         stop=(c == DC // 2 - 1))
                    out_cb(j, ps)

            def k_out(j, ps):
                t = kvoutp.tile([128, TOK], f8, tag="kvout")
                nc.vector.tensor_scalar(t[:], ps[:], 1.0 / 32.0, None,
                                        op0=ALU.mult)
                half, jh = j // 4, j % 4
                nc.sync.dma_start(k_own[half][128 * jh:128 * (jh + 1), :], t[:])
                if jh == 3:
                    nc.gpsimd.collective_compute(
                        "AllGather", ALU.bypass, ins=[k_own[half].opt()],
                        outs=[k_ag[half].opt()], replica_groups=RG2)

            q_sb = []

            def q_out(j, ps):
                t = qp.tile([128, TOK], f8, tag=f"q{j}")
                nc.vector.tensor_scalar(t[:], ps[:], 1.0 / 32.0, None,
                                        op0=ALU.mult)
                q_sb.append(t)

            qk_gemm(1, k_out)

            # vT GEMM: out[tok, feat] with x tiles stationary (DoubleRow)
            wv_sb = []
            for c in range(DC // 2):
                wvt = wvp.tile([128, 2, D], f8, tag=f"wv{c}")
                nc.sync.dma_start(wvt[:], wvr[c])
                wv_sb.append(wvt)
            for t4 in range(TOK // 128):
                vt_sb = vtp.tile([128, D], f8, tag="vt")
                for half in range(2):
                    vps = psp.tile([128, TOK], f32, tag="ps")
                    for c in range(DC // 2):
                        nc.tensor.matmul(
                            vps[:],
                            x3n[c][:, :, 128 * t4:128 * (t4 + 1)],
                            wv_sb[c][:, :, 512 * half:512 * (half + 1)],
                            perf_mode=DR,
                            start=(c == 0), stop=(c == DC // 2 - 1))
                    nc.vector.tensor_scalar(
                        vt_sb[:, 512 * half:512 * (half + 1)], vps[:],
                        1.0 / 32.0, None, op0=ALU.mult)
                for half in range(2):
                    nc.sync.dma_start(
                        v_ownT[half][128 * t4:128 * (t4 + 1), :],
                        vt_sb[:, 512 * half:512 * (half + 1)])
            for half in range(2):
                nc.gpsimd.collective_compute(
                    "AllGather", ALU.bypass, ins=[v_ownT[half].opt()],
                    outs=[v_agT[half].opt()], replica_groups=RG2)

            qk_gemm(0, q_out)

            p1s.close()

            # ---- residual-phase pools (live until kernel end) ----
            ress = ExitStack()
            owp = ress.enter_context(tc.tile_pool(name="owp", bufs=1))
            aop = ress.enter_context(tc.tile_pool(name="aop", bufs=1))
            resbp = ress.enter_context(tc.tile_pool(name="resbp", bufs=1))

            ow_sb = []
            for oc in range(DC):
                t = owp.tile([128, DC, 128], f8, tag=f"ow{oc}")
                nc.sync.dma_start(t[:], owblk[oc])
                ow_sb.append(t)
            xff = xb  # residual reuses the P1-resident x tiles

            # ctxT destination tiles (written via SBUF->SBUF DMA bounce),
            # fp8 pairs for the DoubleRow attn_ow matmul, scaled x64
            ctxT3 = [ctxp.tile([128, 2, TOK], f8, tag=f"ctxT3{c}",
                               name=f"ctxT3{c}") for c in range(DC // 2)]

            # ================= P2: attention, head pairs =================
            p2s = ExitStack()
            kvp = p2s.enter_context(tc.tile_pool(name="kvp", bufs=8))
            vaugp = p2s.enter_context(tc.tile_pool(name="vaugp", bufs=18))
            expp = p2s.enter_context(tc.tile_pool(name="expp", bufs=18))
            expd = p2s.enter_context(tc.tile_pool(name="expd", bufs=6))
            rbp = p2s.enter_context(tc.tile_pool(name="rbp", bufs=2))
            cnp = p2s.enter_context(tc.tile_pool(name="cnp", bufs=4))
            cup = p2s.enter_context(tc.tile_pool(name="cup", bufs=4))

            # pre-touch vaug buffers once so the two ones-columns persist
            # across pool rotations (DMA below only writes the v slices).
            # layout [128 keys, 2(kc pair), 160]: head0 v at 0:64, ones at
            # 64; head1 v at 80:144, ones at 144 (offsets %16 for DR).
            for _ in range(18):
                va = vaugp.tile([128, 2, 160], f8, tag="vaug")
                for ko in range(2):
                    nc.gpsimd.memset(va[:, ko, 64:65], 1.0)
                    nc.gpsimd.memset(va[:, ko, 144:145], 1.0)

            def load_k_tiles(p):
                half, ph = p // 4, p % 4
                ks = []
                for r in range(NR):
                    row0 = HD2 * r + 128 * ph
                    kt = kvp.tile([128, TOK], f8, tag="kv")
                    nc.sync.dma_start(kt[:], k_ag[half][row0:row0 + 128, :])
                    ks.append(kt)
                return ks

            def load_va_tiles(p):
                half, ph = p // 4, p % 4
                vag = v_agT[half]
                vas = []
                for jj in range(NJJ):
                    va = vaugp.tile([128, 2, 160], f8, tag="vaug")
                    for ko in range(2):
                        r0 = 256 * jj + 128 * ko
                        nc.sync.dma_start(
                            va[:, ko, 0:64],
                            vag[r0:r0 + 128, 128 * ph:128 * ph + 64])
                        nc.sync.dma_start(
                            va[:, ko, 80:144],
                            vag[r0:r0 + 128, 128 * ph + 64:128 * ph + 128])
                    vas.append(va)
                return vas

            pending = []

            MAGIC = 0x7EF311C3  # fp32 reciprocal bit-trick constant

            def flush_normalize(limit=99):
                # deferred per-head softmax denominator + normalize, fully
                # on DVE (bit-trick reciprocal + one Newton step) so ACT
                # never has to swap away from the exp table set.
                n = 0
                while pending and n < limit:
                    n += 1
                    pp, h, cu = pending.pop(0)
                    # the whole Newton-reciprocal chain runs on the (idle)
                    # GpSimd engine: SBUF-only rows, keeps DVE free for the
                    # fast-exp + normalize work
                    x0 = rbp.tile([128, TOK], f32, tag="rr")
                    # x0 = bitcast(MAGIC - bits(d)) = bits*-1 + MAGIC
                    nc.vector.tensor_scalar(
                        x0[64:65, :].bitcast(mybir.dt.int32),
                        cu[64:65, :].bitcast(mybir.dt.int32),
                        -1, MAGIC, op0=ALU.mult, op1=ALU.add)
                    tt = rbp.tile([128, TOK], f32, tag="tt")
                    nc.gpsimd.tensor_tensor(tt[64:65, :], cu[64:65, :],
                                            x0[64:65, :], op=ALU.mult)
                    # rbn = (t - 2) * x0 = -1/d after one Newton step
                    rbn = rbp.tile([128, TOK], bf16, tag="rbf")
                    nc.vector.scalar_tensor_tensor(rbn[64:65, :], tt[64:65, :],
                                                   2.0, x0[64:65, :],
                                                   op0=ALU.subtract,
                                                   op1=ALU.mult)
                    rbps = psp.tile([64, TOK], f32, tag="ps")
                    nc.tensor.matmul(rbps[:], ones_all[64:65, :],
                                     rbn[64:65, :], start=True, stop=True)
                    cn = cnp.tile([64, TOK], f8, tag="cn")
                    nc.vector.scalar_tensor_tensor(cn[:], cu[0:64, :], -64.0,
                                                   rbps[:], op0=ALU.mult,
                                                   op1=ALU.mult)
                    nc.sync.dma_start(
                        ctxT3[pp // 2][64 * h:64 * h + 64, pp % 2, :], cn[:])

            k_tiles = load_k_tiles(0)
            va_tiles = load_va_tiles(0)
            for p in range(H // 2):
                ks = k_tiles
                vas = va_tiles
                if p + 1 < H // 2:
                    k_tiles = load_k_tiles(p + 1)
                    va_tiles = load_va_tiles(p + 1)

                cps = [psp.tile([65, TOK], f32, tag="ps", name=f"cps{p}_{hh}")
                       for hh in range(2)]
                exps = [None] * NJJ

                def ctx_jj(jj):
                    for h in range(2):
                        nc.tensor.matmul(
                            cps[h][:], vas[jj][:, :, 80 * h:80 * h + 65],
                            exps[jj][:, :, TOK * h:TOK * (h + 1)],
                            perf_mode=DR,
                            start=(jj == 0), stop=(jj == NJJ - 1))

                for jj in range(NJJ):
                    # flush split in two so the DVE FIFO never builds a
                    # backlog in front of the Schraudolph jj's
                    if jj in (1, 4) and pending:
                        flush_normalize(1)
                    dve_jj = jj in (3, 6)
                    # Schraudolph jj's are written ENTIRELY by DVE into
                    # their own pool, so ACT and DVE never share a tile
                    # (tile-granular ordering would stall ACT's FIFO)
                    e2 = (expd if dve_jj else expp).tile(
                        [128, 2, 2 * TOK], f8,
                        tag="expd" if dve_jj else "exp")
                    for ko in range(2):
                        kc = 2 * jj + ko
                        r, cj = kc // (TOK // 128), kc % (TOK // 128)
                        csl = slice(128 * cj, 128 * (cj + 1))
                        sps = psB.tile([128, 2 * TOK], f32, tag="ps2")
                        nc.tensor.matmul(sps[:, 0:TOK], ks[r][0:64, csl],
                                         q_sb[p][0:64, :],
                                         start=True, stop=True)
                        nc.tensor.matmul(sps[:, TOK:2 * TOK], ks[r][64:128, csl],
                                         q_sb[p][64:128, :],
                                         start=True, stop=True)
                        if dve_jj:
                            # fast-exp straight into fp8e4m3 bits: bits =
                            # round(8*(log2e*s/8 + 7) - 0.336)
                            nc.vector.tensor_scalar(
                                e2[:, ko, :].bitcast(mybir.dt.uint8), sps[:],
                                1.4426950408889634, 55.664,
                                op0=ALU.mult, op1=ALU.add)
                        else:
                            nc.scalar.activation(e2[:, ko, :], sps[:], AF.Exp,
                                                 scale=1.0 / np.sqrt(HD))
                    exps[jj] = e2
                    # lag ctx TWO steps behind the scores/exp so the
                    # in-order PE queue never waits on ACT: by the time PE
                    # reaches ctx(jj-2), its exp finished two slots ago
                    if jj > 1:
                        ctx_jj(jj - 2)
                ctx_jj(NJJ - 2)
                ctx_jj(NJJ - 1)

                # drain ctx+denominator to SBUF; the denominator recip and
                # PE-side broadcast + normalize are deferred into the next
                # pair so they never stall ACT at the pair boundary
                for h in range(2):
                    cu = cup.tile([65, TOK], f32, tag="cu", name=f"cu{p}_{h}")
                    nc.vector.tensor_copy(cu[:], cps[h][:])
                    pending.append((p, h, cu))

            flush_normalize()
            p2s.close()

            # ================= P3: attn_ow + residual + LN2 =================
            # ao (x64) and normalized resid go into fp8 [128,2,TOK] pair
            # tiles for the DoubleRow MLP matmuls
            ao3 = [aop.tile([128, 2, TOK], f8, tag=f"ao3{c}", name=f"ao3{c}")
                   for c in range(DC // 2)]
            res3 = [resbp.tile([128, 2, TOK], f8, tag=f"res3{c}", name=f"res3{c}")
                    for c in range(DC // 2)]
            res_b = []
            for oc in range(DC):
                pps = psp.tile([128, TOK], f32, tag="ps")
                for c in range(DC // 2):
                    nc.tensor.matmul(pps[:], ow_sb[oc][:, 2 * c:2 * c + 2, :],
                                     ctxT3[c][:], perf_mode=DR,
                                     start=(c == 0), stop=(c == DC // 2 - 1))
                # psum carries 64*32 = 2048x scale from ctx and ow fp8
                nc.vector.tensor_scalar(ao3[oc // 2][:, oc % 2, :], pps[:],
                                        1.0 / 32.0, None, op0=ALU.mult)
                rb = resbp.tile([128, TOK], bf16, tag=f"resb{oc}")
                nc.vector.scalar_tensor_tensor(rb[:], pps[:], 1.0 / 2048.0,
                                               xff[oc][:], op0=ALU.mult,
                                               op1=ALU.add)
                res_b.append(rb)

            rstd2, mean2 = ln_stats(lambda d: res_b[d][:])
            for oc in range(DC):
                # split the normalize across DVE and GpSimd so the serial
                # LN2 -> res3 chain in front of the MLP halves in length
                eng = nc.vector if oc % 2 == 0 else nc.gpsimd
                rc = wfp.tile([128, TOK], bf16, tag="wf")
                eng.tensor_tensor(rc[:], res_b[oc][:], mean2[:],
                                  op=ALU.subtract)
                eng.tensor_tensor(res3[oc // 2][:, oc % 2, :],
                                  rc[:], rstd2[:], op=ALU.mult)

            # ================= P4: MLP =================
            p4s = ExitStack()
            intp = p4s.enter_context(tc.tile_pool(name="intp", bufs=1))
            h1p = p4s.enter_context(tc.tile_pool(name="h1p", bufs=3))
            w1sp = p4s.enter_context(tc.tile_pool(name="w1sp", bufs=4))
            w2sp = p4s.enter_context(tc.tile_pool(name="w2sp", bufs=4))
            outwsp = p4s.enter_context(tc.tile_pool(name="outwsp", bufs=1))
            ofp = p4s.enter_context(tc.tile_pool(name="ofp", bufs=3))
            ofhp = p4s.enter_context(tc.tile_pool(name="ofhp", bufs=1))

            inter3 = [intp.tile([128, 2, TOK], f8, tag=f"it3{i}", name=f"it3{i}")
                      for i in range(IC // 2)]
            ow_tiles = []
            for oc in range(DC):
                owt = outwsp.tile([128, IC, 128], f8, tag=f"outws{oc}")
                nc.sync.dma_start(owt[:], outwblk[oc])
                ow_tiles.append(owt)
            of_half = []
            for ic in range(IC):
                # w2 GEMM first: it only needs ao3, so it fills the PE while
                # the LN2 chain (stats -> rstd2 -> res3) is still running
                w2t = w2sp.tile([128, DC, 128], f8, tag="w2s")
                nc.sync.dma_start(w2t[:], w2blk[ic])
                h2ps = psp.tile([128, TOK], f32, tag="ps")
                for c in range(DC // 2):
                    nc.tensor.matmul(h2ps[:], w2t[:, 2 * c:2 * c + 2, :],
                                     ao3[c][:], perf_mode=DR,
                                     start=(c == 0), stop=(c == DC // 2 - 1))

                w1t = w1sp.tile([128, DC, 128], f8, tag="w1s")
                nc.sync.dma_start(w1t[:], w1blk[ic])
                h1ps = psB.tile([128, TOK], f32, tag="ps2")
                for c in range(DC // 2):
                    nc.tensor.matmul(h1ps[:], w1t[:, 2 * c:2 * c + 2, :],
                                     res3[c][:], perf_mode=DR,
                                     start=(c == 0), stop=(c == DC // 2 - 1))
                h1 = h1p.tile([128, TOK], bf16, tag="h1")
                nc.scalar.activation(h1[:], h1ps[:], AF.Gelu, scale=1.0 / 32.0)
                nc.vector.tensor_tensor(inter3[ic // 2][:, ic % 2, :],
                                        h2ps[:], h1[:], op=ALU.mult)

                if ic == IC // 2 + 1:
                    # first-half output projection: frees the tail from a
                    # serial 128-MM + 4MB-DMA block after the last inter3
                    for oc in range(DC):
                        ops = psp.tile([128, TOK], f32, tag="ps")
                        for i in range(IC // 4):
                            nc.tensor.matmul(
                                ops[:], ow_tiles[oc][:, 2 * i:2 * i + 2, :],
                                inter3[i][:], perf_mode=DR,
                                start=(i == 0), stop=(i == IC // 4 - 1))
                        ofh = ofhp.tile([128, TOK], f32, tag=f"ofh{oc}")
                        # partial/65536 + resid
                        nc.vector.scalar_tensor_tensor(
                            ofh[:], ops[:], 1.0 / 65536.0, res_b[oc][:],
                            op0=ALU.mult, op1=ALU.add)
                        of_half.append(ofh)

            for oc in range(DC):
                ops = psp.tile([128, TOK], f32, tag="ps")
                for i in range(IC // 4, IC // 2):
                    nc.tensor.matmul(ops[:], ow_tiles[oc][:, 2 * i:2 * i + 2, :],
                                     inter3[i][:], perf_mode=DR,
                                     start=(i == IC // 4),
                                     stop=(i == IC // 2 - 1))
                of = ofp.tile([128, TOK], f32, tag="of")
                nc.vector.scalar_tensor_tensor(of[:], ops[:], 1.0 / 65536.0,
                                               of_half[oc][:], op0=ALU.mult,
                                               op1=ALU.add)
                nc.sync.dma_start(outT[128 * oc:128 * (oc + 1), :], of[:])

            p4s.close()
            ress.close()

    nc.compile()
    return nc


_NC_CACHE = {}


def kernel(**inputs):
    x = np.asarray(inputs["x"], np.float32)
    norm_w = np.asarray(inputs["norm_w"], np.float32)
    norm_b = np.asarray(inputs["norm_b"], np.float32)
    qkvw = np.asarray(inputs["attn_qkvw"], np.float32)
    qkvb = np.asarray(inputs["attn_qkvb"], np.float32)
    attn_ow = np.asarray(inputs["attn_ow"], np.float32)
    attn_ob = np.asarray(inputs["attn_ob"], np.float32)
    attn_nw = np.asarray(inputs["attn_nw"], np.float32)
    attn_nb = np.asarray(inputs["attn_nb"], np.float32)
    inter_w = np.asarray(inputs["inter_w"], np.float32)
    inter_b = np.asarray(inputs["inter_b"], np.float32)
    inter_w1 = np.asarray(inputs["inter_w1"], np.float32)
    output_w = np.asarray(inputs["output_w"], np.float32)
    output_b = np.asarray(inputs["output_b"], np.float32)

    X = x.reshape(T, D)
    XT = np.ascontiguousarray(X.T)          # [D, T]

    # ---- LN folds (host) ----
    wqkv_f = norm_w[:, None] * qkvw          # [D, 3D]
    bqkv_f = qkvb + norm_b @ qkvw

    w1_f = attn_nw[:, None] * inter_w        # [D, I]
    b1_f = inter_b + attn_nb @ inter_w

    assert not np.any(bqkv_f) and not np.any(attn_ob) and not np.any(b1_f) \
        and not np.any(output_b), "nonzero biases not wired in this build"

    wq_s = 32.0 * wqkv_f
    wqblk = _f8(wq_s.reshape(DC, 128, 3, DC, 128).transpose(2, 3, 1, 0, 4))
    # v weights in [d, feat] orientation, DoubleRow chunk-pairs
    wv_s = wq_s[:, 2 * D:3 * D]              # [D, D]
    wvr = _f8(wv_s.reshape(DC // 2, 2, 128, D).transpose(0, 2, 1, 3))
    owblk = _f8((32.0 * attn_ow).reshape(DC, 128, DC, 128).transpose(2, 1, 0, 3))
    w1_s = 32.0 * w1_f
    w1blk = _f8(w1_s.reshape(DC, 128, IC, 128).transpose(2, 1, 0, 3))
    w2blk = _f8((32.0 * inter_w1).reshape(DC, 128, IC, 128).transpose(2, 1, 0, 3))
    outwblk = _f8((32.0 * output_w).reshape(IC, 128, DC, 128)
                  .transpose(2, 1, 0, 3))

    if "nc" not in _NC_CACHE:
        _NC_CACHE["nc"] = _build()
    nc = _NC_CACHE["nc"]

    in_maps = []
    for c in range(NC):
        tsl = slice(TOK * c, TOK * (c + 1))
        xs = np.ascontiguousarray(XT[:, tsl])
        in_maps.append({
            "xbf": _bf16(xs),
            "wqblk": wqblk,
            "wvr": wvr,
            "owblk": owblk,
            "w1blk": w1blk,
            "w2blk": w2blk,
            "outwblk": outwblk,
        })

    global _LAST_IN_MAPS
    _LAST_IN_MAPS = in_maps
    res = run_bass_kernel_spmd(nc, in_maps, list(range(NC)))
    outT = np.concatenate([res.results[c]["outT"] for c in range(NC)], axis=1)
    return np.ascontiguousarray(outT.T).reshape(B, S, D).astype(np.float32)


if __name__ == "__main__":
    pass


# revision 58
# speedup vs baseline: 1.0718x; 1.0718x over previous
"""Sequence-parallel DeepSpeed encoder-decoder block on 8 TRN2 NeuronCores.

Each core owns a 512-token block (cores 0-3 -> sequence 0, cores 4-7 ->
sequence 1) and computes the FULL layer for its tokens: LN1 + QKV (all 16
heads), attention against the whole sequence, attn_ow, residual, LN2, and
the complete MLP. The only communication is an AllGather of k and v^T
within each sequence's 4-core group; the final output is token-sharded so
the host just concatenates - no ReduceScatter at all.

v2 changes vs the first working kernel:
- A tiny warm-up AllGather issued at t=0 absorbs the ~40us collective
  rendezvous behind the P1 GEMMs instead of stalling attention.
- V is produced directly in transposed layout (DoubleRow GEMM with the
  x tiles stationary), AllGathered as [tokens, feats]; the per-pair PE
  transposes + PSUM drains of v1 are gone.  The augmented ones columns
  (softmax denominator) persist in SBUF; DMA fills only the v slices.
- exp() is written as fp8e4 in a DoubleRow-paired layout, so the ctx
  matmuls run DoubleRow (half the instructions of v1).
- LayerNorm rstd is folded INTO the fp8 activation tiles (x3n/res3n),
  so every QKV/MLP PSUM drain is a plain tensor_scalar instead of a
  tensor_tensor against the rstd broadcast; 1/std comes from exp(-.5 ln)
  on two [1,512] rows (same ACT table set as attention) instead of the
  slow DVE reciprocal on [128,512].
"""
from contextlib import ExitStack

import numpy as np
import ml_dtypes

import concourse.bacc as bacc
import concourse.mybir as mybir
import concourse.tile as tile
from concourse.bass_utils import run_bass_kernel_spmd

f32 = mybir.dt.float32
bf16 = mybir.dt.bfloat16
f8 = mybir.dt.float8e4
AF = mybir.ActivationFunctionType
ALU = mybir.AluOpType

NC = 8
B, S, D, I = 2, 2048, 1024, 4096
H, HD = 16, 64
T = B * S
TOK = T // NC        # 512 tokens per core
DC = D // 128        # 8 feature chunks
IC = I // 128        # 32 intermediate chunks
NKC = S // 128       # 16 key chunks per sequence
NJJ = NKC // 2       # 8 key-pair chunks (DoubleRow ctx granularity)
NR = 4               # ranks per sequence group
EPS = 1e-12

_BF = ml_dtypes.bfloat16


def _bf16(a):
    return np.ascontiguousarray(a.astype(_BF))


_F8 = ml_dtypes.float8_e4m3


def _f8(a):
    return np.ascontiguousarray(a.astype(_F8))


def _build():
    nc = bacc.Bacc("TRN2", target_bir_lowering=False, debug=False, num_devices=NC)

    inp = {}
    def din(name, shape, dt):
        inp[name] = nc.dram_tensor(name, shape, dt, kind="ExternalInput")
        return inp[name]

    xbf = din("xbf", [D, TOK], bf16)
    wqblk = din("wqblk", [3, DC, 128, DC, 128], f8)
    wvr = din("wvr", [DC // 2, 128, 2, D], f8)
    owblk = din("owblk", [DC, 128, DC, 128], f8)
    w1blk = din("w1blk", [IC, 128, DC, 128], f8)
    w2blk = din("w2blk", [IC, 128, DC, 128], f8)
    outwblk = din("outwblk", [DC, 128, IC, 128], f8)

    outT = nc.dram_tensor("outT", [D, TOK], f32, kind="ExternalOutput")

    RG2 = [[0, 1, 2, 3], [4, 5, 6, 7]]

    with tile.TileContext(nc) as tc:
        with ExitStack() as ctx:
            ep = ctx.enter_context
            # ---- global pools (whole-kernel lifetime) ----
            cons = ep(tc.tile_pool(name="cons", bufs=1))
            qp = ep(tc.tile_pool(name="qp", bufs=1))
            ctxp = ep(tc.tile_pool(name="ctxp", bufs=1))
            rowp = ep(tc.tile_pool(name="rowp", bufs=10))
            xresp = ep(tc.tile_pool(name="xresp", bufs=1))
            rstdp = ep(tc.tile_pool(name="rstdp", bufs=2))
            wfp = ep(tc.tile_pool(name="wfp", bufs=6))
            psp = ep(tc.tile_pool(name="psp", bufs=4, space="PSUM"))
            psB = ep(tc.tile_pool(name="psB", bufs=2, space="PSUM"))
            dram = ep(tc.tile_pool(name="dram", bufs=1, space="DRAM"))

            # DRAM scratch for the kv AllGathers, split into feature halves
            # so attention on head-pairs 0-3 can start after the first
            # quarter of the (serialized, BW-bound) collective chain.
            HD2 = D // 2
            k_own = [dram.tile([HD2, TOK], f8, tag=f"k_own{i}", name=f"k_own{i}")
                     for i in range(2)]
            v_ownT = [dram.tile([TOK, HD2], f8, tag=f"v_ownT{i}",
                                name=f"v_ownT{i}") for i in range(2)]
            k_ag = [dram.tile([NR * HD2, TOK], f8, tag=f"k_ag{i}",
                              name=f"k_ag{i}") for i in range(2)]
            v_agT = [dram.tile([NR * TOK, HD2], f8, tag=f"v_agT{i}",
                               name=f"v_agT{i}") for i in range(2)]

            # ---- constants ----
            ones_col = cons.tile([128, 1], bf16)
            nc.gpsimd.memset(ones_col[:], 1.0)
            ones_all = cons.tile([128, 64], bf16)
            nc.gpsimd.memset(ones_all[:], 1.0)
            ones_row128 = cons.tile([1, 128], bf16)
            nc.gpsimd.memset(ones_row128[:], 1.0)
            invD_f = cons.tile([1, 128], f32)
            nc.gpsimd.memset(invD_f[:], 1.0 / D)
            invD_row = cons.tile([1, 128], bf16)
            nc.vector.tensor_copy(invD_row[:], invD_f[:])
            eps_one = cons.tile([1, 1], f32)
            nc.gpsimd.memset(eps_one[:], EPS)



            RMAGIC = 0x5F3759DF  # fp32 rsqrt bit-trick constant

            def ln_stats(feed_tile_fn):
                """feed_tile_fn(d) -> bf16 [128,TOK] AP.  Returns
                (rstd_bc [128,TOK] f32 SBUF broadcast, mt_row [1,TOK] bf16
                = mean*rstd).  1/sqrt(var) runs entirely on DVE (bit-trick
                + one Newton step) so ACT's exp table set stays resident."""
                sum_ps = psp.tile([1, TOK], f32, tag="ps")
                ssq_ps = psp.tile([1, TOK], f32, tag="ps")
                for d in range(DC):
                    xt = feed_tile_fn(d)
                    sq = wfp.tile([128, TOK], bf16, tag="wf")
                    nc.vector.tensor_tensor(sq[:], xt, xt, op=ALU.mult)
                    nc.tensor.matmul(sum_ps[:], ones_col[:], xt,
                                     start=(d == 0), stop=(d == DC - 1))
                    nc.tensor.matmul(ssq_ps[:], ones_col[:], sq[:],
                                     start=(d == 0), stop=(d == DC - 1))
                mean_r = rowp.tile([1, TOK], f32, tag="rowf")
                nc.vector.tensor_scalar(mean_r[:], sum_ps[:], 1.0 / D, None,
                                        op0=ALU.mult)
                msq_r = rowp.tile([1, TOK], f32, tag="rowf")
                nc.vector.tensor_tensor(msq_r[:], mean_r[:], mean_r[:],
                                        op=ALU.mult)
                var_r = rowp.tile([1, TOK], f32, tag="rowf")
                nc.vector.scalar_tensor_tensor(var_r[:], ssq_ps[:], 1.0 / D,
                                               msq_r[:], op0=ALU.mult,
                                               op1=ALU.subtract)
                # rstd = rsqrt(var): x0 = bitcast(RMAGIC - (bits >> 1));
                # rstd = x0 * (1.5 - 0.5 * var * x0^2)
                sh_r = rowp.tile([1, TOK], f32, tag="rowf")
                nc.vector.tensor_scalar(sh_r[:].bitcast(mybir.dt.int32),
                                        var_r[:].bitcast(mybir.dt.int32),
                                        1, None, op0=ALU.arith_shift_right)
                x0_r = rowp.tile([1, TOK], f32, tag="rowf")
                nc.vector.tensor_scalar(x0_r[:].bitcast(mybir.dt.int32),
                                        sh_r[:].bitcast(mybir.dt.int32),
                                        -1, RMAGIC, op0=ALU.mult, op1=ALU.add)
                s1_r = rowp.tile([1, TOK], f32, tag="rowf")
                nc.vector.tensor_tensor(s1_r[:], x0_r[:], x0_r[:], op=ALU.mult)
                s2_r = rowp.tile([1, TOK], f32, tag="rowf")
                nc.vector.tensor_tensor(s2_r[:], var_r[:], s1_r[:], op=ALU.mult)
                w_r = rowp.tile([1, TOK], f32, tag="rowf")
                nc.vector.tensor_scalar(w_r[:], s2_r[:], -0.5, 1.5,
                                        op0=ALU.mult, op1=ALU.add)
                rstd_r = rowp.tile([1, TOK], bf16, tag="row")
                nc.vector.tensor_tensor(rstd_r[:], w_r[:], x0_r[:], op=ALU.mult)
                # broadcast 1/std and mean to all 128 partitions via PE
                bc_ps = psp.tile([128, TOK], f32, tag="ps")
                nc.tensor.matmul(bc_ps[:], ones_row128[:], rstd_r[:],
                                 start=True, stop=True)
                rstd_bc = rstdp.tile([128, TOK], f32, tag="rstd")
                nc.vector.tensor_copy(rstd_bc[:], bc_ps[:])
                mean_b = rowp.tile([1, TOK], bf16, tag="row")
                nc.vector.tensor_copy(mean_b[:], mean_r[:])
                mb_ps = psp.tile([128, TOK], f32, tag="ps")
                nc.tensor.matmul(mb_ps[:], ones_row128[:], mean_b[:],
                                 start=True, stop=True)
                mean_bc = rstdp.tile([128, TOK], f32, tag="mbc")
                nc.vector.tensor_copy(mean_bc[:], mb_ps[:])
                return rstd_bc, mean_bc

            # ================= P1: LN1 + QKV (k, vT, then q) =================
            p1s = ExitStack()
            wqp = p1s.enter_context(tc.tile_pool(name="wqp", bufs=6))
            xbp = p1s.enter_context(tc.tile_pool(name="xbp", bufs=1))
            kvoutp = p1s.enter_context(tc.tile_pool(name="kvoutp", bufs=4))
            wvp = p1s.enter_context(tc.tile_pool(name="wvp", bufs=4))
            vtp = p1s.enter_context(tc.tile_pool(name="vtp", bufs=2))

            xb = []
            for d in range(DC):
                t = xresp.tile([128, TOK], bf16, tag=f"xb{d}")
                nc.sync.dma_start(t[:], xbf[128 * d:128 * (d + 1), :])
                xb.append(t)

            rstd1, mean1 = ln_stats(lambda d: xb[d][:])

            # fully normalized fp8 x tiles ((x-mean)*rstd), DoubleRow pairs:
            # no rank-1 mean-correction matmuls needed anywhere downstream
            x3n = [xbp.tile([128, 2, TOK], f8, tag=f"x3n{c}", name=f"x3n{c}")
                   for c in range(DC // 2)]
            for d in range(DC):
                xc = wfp.tile([128, TOK], bf16, tag="wf")
                nc.vector.tensor_tensor(xc[:], xb[d][:], mean1[:],
                                        op=ALU.subtract)
                nc.vector.tensor_tensor(x3n[d // 2][:, d % 2, :], xc[:],
                                        rstd1[:], op=ALU.mult)

            DR = mybir.MatmulPerfMode.DoubleRow

            # ---- k then q GEMMs (fp8 DR), vT GEMM in between ----
            def qk_gemm(part, out_cb):
                for j in range(DC):
                    wqt = wqp.tile([128, DC, 128], f8, tag="wqs")
                    nc.sync.dma_start(wqt[:], wqblk[part, j])
                    ps = psp.tile([128, TOK], f32, tag="ps")
                    for c in range(DC // 2):
                        nc.tensor.matmul(ps[:], wqt[:, 2 * c:2 * c + 2, :],
                                         x3n[c][:], perf_mode=DR,
                                         start=(c == 0),
                                         stop=(c == DC // 2 - 1))
                    out_cb(j, ps)

            def k_out(j, ps):
                t = kvoutp.tile([128, TOK], f8, tag="kvout")
                nc.vector.tensor_scalar(t[:], ps[:], 1.0 / 32.0, None,
                                        op0=ALU.mult)
                half, jh = j // 4, j % 4
                nc.sync.dma_start(k_own[half][128 * jh:128 * (jh + 1), :], t[:])
                if jh == 3:
                    nc.gpsimd.collective_compute(
                        "AllGather", ALU.bypass, ins=[k_own[half].opt()],
                        outs=[k_ag[half].opt()], replica_groups=RG2)

            q_sb = []

            def q_out(j, ps):
                t = qp.tile([128, TOK], f8, tag=f"q{j}")
                nc.vector.tensor_scalar(t[:], ps[:], 1.0 / 32.0, None,
                                        op0=ALU.mult)
                q_sb.append(t)

            qk_gemm(1, k_out)

            # vT GEMM: out[tok, feat] with x tiles stationary (DoubleRow)
            wv_sb = []
            for c in range(DC // 2):
                wvt = wvp.tile([128, 2, D], f8, tag=f"wv{c}")
                nc.sync.dma_start(wvt[:], wvr[c])
                wv_sb.append(wvt)
            for t4 in range(TOK // 128):
                vt_sb = vtp.tile([128, D], f8, tag="vt")
                for half in range(2):
                    vps = psp.tile([128, TOK], f32, tag="ps")
                    for c in range(DC // 2):
                        nc.tensor.matmul(
                            vps[:],
                            x3n[c][:, :, 128 * t4:128 * (t4 + 1)],
                            wv_sb[c][:, :, 512 * half:512 * (half + 1)],
                            perf_mode=DR,
                            start=(c == 0), stop=(c == DC // 2 - 1))
                    nc.vector.tensor_scalar(
                        vt_sb[:, 512 * half:512 * (half + 1)], vps[:],
                        1.0 / 32.0, None, op0=ALU.mult)
                for half in range(2):
                    nc.sync.dma_start(
                        v_ownT[half][128 * t4:128 * (t4 + 1), :],
                        vt_sb[:, 512 * half:512 * (half + 1)])
            for half in range(2):
                nc.gpsimd.collective_compute(
                    "AllGather", ALU.bypass, ins=[v_ownT[half].opt()],
                    outs=[v_agT[half].opt()], replica_groups=RG2)

            qk_gemm(0, q_out)

            p1s.close()

            # ---- residual-phase pools (live until kernel end) ----
            ress = ExitStack()
            owp = ress.enter_context(tc.tile_pool(name="owp", bufs=1))
            aop = ress.enter_context(tc.tile_pool(name="aop", bufs=1))
            resbp = ress.enter_context(tc.tile_pool(name="resbp", bufs=1))

            ow_sb = []
            for oc in range(DC):
                t = owp.tile([128, DC, 128], f8, tag=f"ow{oc}")
                nc.sync.dma_start(t[:], owblk[oc])
                ow_sb.append(t)
            xff = xb  # residual reuses the P1-resident x tiles

            # ctxT destination tiles (written via SBUF->SBUF DMA bounce),
            # fp8 pairs for the DoubleRow attn_ow matmul, scaled x64
            ctxT3 = [ctxp.tile([128, 2, TOK], f8, tag=f"ctxT3{c}",
                               name=f"ctxT3{c}") for c in range(DC // 2)]

            # ================= P2: attention, head pairs =================
            p2s = ExitStack()
            kvp = p2s.enter_context(tc.tile_pool(name="kvp", bufs=8))
            vaugp = p2s.enter_context(tc.tile_pool(name="vaugp", bufs=18))
            expp = p2s.enter_context(tc.tile_pool(name="expp", bufs=18))
            expd = p2s.enter_context(tc.tile_pool(name="expd", bufs=6))
            rbp = p2s.enter_context(tc.tile_pool(name="rbp", bufs=2))
            cnp = p2s.enter_context(tc.tile_pool(name="cnp", bufs=4))
            cup = p2s.enter_context(tc.tile_pool(name="cup", bufs=4))

            # pre-touch vaug buffers once so the two ones-columns persist
            # across pool rotations (DMA below only writes the v slices).
            # layout [128 keys, 2(kc pair), 160]: head0 v at 0:64, ones at
            # 64; head1 v at 80:144, ones at 144 (offsets %16 for DR).
            for _ in range(18):
                va = vaugp.tile([128, 2, 160], f8, tag="vaug")
                for ko in range(2):
                    nc.gpsimd.memset(va[:, ko, 64:65], 1.0)
                    nc.gpsimd.memset(va[:, ko, 144:145], 1.0)

            def load_k_tiles(p):
                half, ph = p // 4, p % 4
                ks = []
                for r in range(NR):
                    row0 = HD2 * r + 128 * ph
                    kt = kvp.tile([128, TOK], f8, tag="kv")
                    nc.sync.dma_start(kt[:], k_ag[half][row0:row0 + 128, :])
                    ks.append(kt)
                return ks

            def load_va_tiles(p):
                half, ph = p // 4, p % 4
                vag = v_agT[half]
                vas = []
                for jj in range(NJJ):
                    va = vaugp.tile([128, 2, 160], f8, tag="vaug")
                    for ko in range(2):
                        r0 = 256 * jj + 128 * ko
                        nc.sync.dma_start(
                            va[:, ko, 0:64],
                            vag[r0:r0 + 128, 128 * ph:128 * ph + 64])
                        nc.sync.dma_start(
                            va[:, ko, 80:144],
                            vag[r0:r0 + 128, 128 * ph + 64:128 * ph + 128])
                    vas.append(va)
                return vas

            pending = []

            MAGIC = 0x7EF311C3  # fp32 reciprocal bit-trick constant

            def flush_normalize(limit=99):
                # deferred per-head softmax denominator + normalize, fully
                # on DVE (bit-trick reciprocal + one Newton step) so ACT
                # never has to swap away from the exp table set.
                n = 0
                while pending and n < limit:
                    n += 1
                    pp, h, cu = pending.pop(0)
                    # the whole Newton-reciprocal chain runs on the (idle)
                    # GpSimd engine: SBUF-only rows, keeps DVE free for the
                    # fast-exp + normalize work
                    x0 = rbp.tile([128, TOK], f32, tag="rr")
                    # x0 = bitcast(MAGIC - bits(d)) = bits*-1 + MAGIC
                    nc.vector.tensor_scalar(
                        x0[64:65, :].bitcast(mybir.dt.int32),
                        cu[64:65, :].bitcast(mybir.dt.int32),
                        -1, MAGIC, op0=ALU.mult, op1=ALU.add)
                    tt = rbp.tile([128, TOK], f32, tag="tt")
                    nc.vector.tensor_tensor(tt[64:65, :], cu[64:65, :],
                                            x0[64:65, :], op=ALU.mult)
                    # rbn = (t - 2) * x0 = -1/d after one Newton step
                    rbn = rbp.tile([128, TOK], bf16, tag="rbf")
                    nc.vector.scalar_tensor_tensor(rbn[64:65, :], tt[64:65, :],
                                                   2.0, x0[64:65, :],
                                                   op0=ALU.subtract,
                                                   op1=ALU.mult)
                    rbps = psp.tile([64, TOK], f32, tag="ps")
                    nc.tensor.matmul(rbps[:], ones_all[64:65, :],
                                     rbn[64:65, :], start=True, stop=True)
                    cn = cnp.tile([64, TOK], f8, tag="cn")
                    nc.vector.scalar_tensor_tensor(cn[:], cu[0:64, :], -64.0,
                                                   rbps[:], op0=ALU.mult,
                                                   op1=ALU.mult)
                    nc.sync.dma_start(
                        ctxT3[pp // 2][64 * h:64 * h + 64, pp % 2, :], cn[:])

            k_tiles = load_k_tiles(0)
            va_tiles = load_va_tiles(0)
            for p in range(H // 2):
                ks = k_tiles
                vas = va_tiles
                if p + 1 < H // 2:
                    k_tiles = load_k_tiles(p + 1)
                    va_tiles = load_va_tiles(p + 1)

                cps = [psp.tile([65, TOK], f32, tag="ps", name=f"cps{p}_{hh}")
                       for hh in range(2)]
                exps = [None] * NJJ

                def ctx_jj(jj):
                    for h in range(2):
                        nc.tensor.matmul(
                            cps[h][:], vas[jj][:, :, 80 * h:80 * h + 65],
                            exps[jj][:, :, TOK * h:TOK * (h + 1)],
                            perf_mode=DR,
                            start=(jj == 0), stop=(jj == NJJ - 1))

                for jj in range(NJJ):
                    # flush split in two so the DVE FIFO never builds a
                    # backlog in front of the Schraudolph jj's
                    if jj in (1, 4) and pending:
                        flush_normalize(1)
                    dve_jj = jj in (3, 6)
                    # Schraudolph jj's are written ENTIRELY by DVE into
                    # their own pool, so ACT and DVE never share a tile
                    # (tile-granular ordering would stall ACT's FIFO)
                    e2 = (expd if dve_jj else expp).tile(
                        [128, 2, 2 * TOK], f8,
                        tag="expd" if dve_jj else "exp")
                    for ko in range(2):
                        kc = 2 * jj + ko
                        r, cj = kc // (TOK // 128), kc % (TOK // 128)
                        csl = slice(128 * cj, 128 * (cj + 1))
                        sps = psB.tile([128, 2 * TOK], f32, tag="ps2")
                        nc.tensor.matmul(sps[:, 0:TOK], ks[r][0:64, csl],
                                         q_sb[p][0:64, :],
                                         start=True, stop=True)
                        nc.tensor.matmul(sps[:, TOK:2 * TOK], ks[r][64:128, csl],
                                         q_sb[p][64:128, :],
                                         start=True, stop=True)
                        if dve_jj:
                            # fast-exp straight into fp8e4m3 bits: bits =
                            # round(8*(log2e*s/8 + 7) - 0.336)
                            nc.vector.tensor_scalar(
                                e2[:, ko, :].bitcast(mybir.dt.uint8), sps[:],
                                1.4426950408889634, 55.664,
                                op0=ALU.mult, op1=ALU.add)
                        else:
                            nc.scalar.activation(e2[:, ko, :], sps[:], AF.Exp,
                                                 scale=1.0 / np.sqrt(HD))
                    exps[jj] = e2
                    # lag ctx TWO steps behind the scores/exp so the
                    # in-order PE queue never waits on ACT: by the time PE
                    # reaches ctx(jj-2), its exp finished two slots ago
                    if jj > 1:
                        ctx_jj(jj - 2)
                ctx_jj(NJJ - 2)
                ctx_jj(NJJ - 1)

                # drain ctx+denominator to SBUF; the denominator recip and
                # PE-side broadcast + normalize are deferred into the next
                # pair so they never stall ACT at the pair boundary
                for h in range(2):
                    cu = cup.tile([65, TOK], f32, tag="cu", name=f"cu{p}_{h}")
                    nc.vector.tensor_copy(cu[:], cps[h][:])
                    pending.append((p, h, cu))

            flush_normalize()
            p2s.close()

            # ================= P3: attn_ow + residual + LN2 =================
            # ao (x64) and normalized resid go into fp8 [128,2,TOK] pair
            # tiles for the DoubleRow MLP matmuls
            ao3 = [aop.tile([128, 2, TOK], f8, tag=f"ao3{c}", name=f"ao3{c}")
                   for c in range(DC // 2)]
            res3 = [resbp.tile([128, 2, TOK], f8, tag=f"res3{c}", name=f"res3{c}")
                    for c in range(DC // 2)]
            res_b = []
            for oc in range(DC):
                pps = psp.tile([128, TOK], f32, tag="ps")
                for c in range(DC // 2):
                    nc.tensor.matmul(pps[:], ow_sb[oc][:, 2 * c:2 * c + 2, :],
                                     ctxT3[c][:], perf_mode=DR,
                                     start=(c == 0), stop=(c == DC // 2 - 1))
                # psum carries 64*32 = 2048x scale from ctx and ow fp8
                nc.vector.tensor_scalar(ao3[oc // 2][:, oc % 2, :], pps[:],
                                        1.0 / 32.0, None, op0=ALU.mult)
                rb = resbp.tile([128, TOK], bf16, tag=f"resb{oc}")
                nc.vector.scalar_tensor_tensor(rb[:], pps[:], 1.0 / 2048.0,
                                               xff[oc][:], op0=ALU.mult,
                                               op1=ALU.add)
                res_b.append(rb)

            rstd2, mean2 = ln_stats(lambda d: res_b[d][:])
            for oc in range(DC):
                rc = wfp.tile([128, TOK], bf16, tag="wf")
                nc.vector.tensor_tensor(rc[:], res_b[oc][:], mean2[:],
                                        op=ALU.subtract)
                nc.vector.tensor_tensor(res3[oc // 2][:, oc % 2, :],
                                        rc[:], rstd2[:], op=ALU.mult)

            # ================= P4: MLP =================
            p4s = ExitStack()
            intp = p4s.enter_context(tc.tile_pool(name="intp", bufs=1))
            h1p = p4s.enter_context(tc.tile_pool(name="h1p", bufs=3))
            w1sp = p4s.enter_context(tc.tile_pool(name="w1sp", bufs=4))
            w2sp = p4s.enter_context(tc.tile_pool(name="w2sp", bufs=4))
            outwsp = p4s.enter_context(tc.tile_pool(name="outwsp", bufs=1))
            ofp = p4s.enter_context(tc.tile_pool(name="ofp", bufs=3))
            ofhp = p4s.enter_context(tc.tile_pool(name="ofhp", bufs=1))

            inter3 = [intp.tile([128, 2, TOK], f8, tag=f"it3{i}", name=f"it3{i}")
                      for i in range(IC // 2)]
            ow_tiles = []
            for oc in range(DC):
                owt = outwsp.tile([128, IC, 128], f8, tag=f"outws{oc}")
                nc.sync.dma_start(owt[:], outwblk[oc])
                ow_tiles.append(owt)
            of_half = []
            for ic in range(IC):
                # w2 GEMM first: it only needs ao3, so it fills the PE while
                # the LN2 chain (stats -> rstd2 -> res3) is still running
                w2t = w2sp.tile([128, DC, 128], f8, tag="w2s")
                nc.sync.dma_start(w2t[:], w2blk[ic])
                h2ps = psp.tile([128, TOK], f32, tag="ps")
                for c in range(DC // 2):
                    nc.tensor.matmul(h2ps[:], w2t[:, 2 * c:2 * c + 2, :],
                                     ao3[c][:], perf_mode=DR,
                                     start=(c == 0), stop=(c == DC // 2 - 1))

                w1t = w1sp.tile([128, DC, 128], f8, tag="w1s")
                nc.sync.dma_start(w1t[:], w1blk[ic])
                h1ps = psB.tile([128, TOK], f32, tag="ps2")
                for c in range(DC // 2):
                    nc.tensor.matmul(h1ps[:], w1t[:, 2 * c:2 * c + 2, :],
                                     res3[c][:], perf_mode=DR,
                                     start=(c == 0), stop=(c == DC // 2 - 1))
                h1 = h1p.tile([128, TOK], bf16, tag="h1")
                nc.scalar.activation(h1[:], h1ps[:], AF.Gelu, scale=1.0 / 32.0)
                nc.vector.tensor_tensor(inter3[ic // 2][:, ic % 2, :],
                                        h2ps[:], h1[:], op=ALU.mult)

                if ic == IC // 2 + 1:
                    # first-half output projection: frees the tail from a
                    # serial 128-MM + 4MB-DMA block after the last inter3
                    for oc in range(DC):
                        ops = psp.tile([128, TOK], f32, tag="ps")
                        for i in range(IC // 4):
                            nc.tensor.matmul(
                                ops[:], ow_tiles[oc][:, 2 * i:2 * i + 2, :],
                                inter3[i][:], perf_mode=DR,
                                start=(i == 0), stop=(i == IC // 4 - 1))
                        ofh = ofhp.tile([128, TOK], f32, tag=f"ofh{oc}")
                        # partial/65536 + resid
                        nc.vector.scalar_tensor_tensor(
                            ofh[:], ops[:], 1.0 / 65536.0, res_b[oc][:],
                            op0=ALU.mult, op1=ALU.add)
                        of_half.append(ofh)

            for oc in range(DC):
                ops = psp.tile([128, TOK], f32, tag="ps")
                for i in range(IC // 4, IC // 2):
                    nc.tensor.matmul(ops[:], ow_tiles[oc][:, 2 * i:2 * i + 2, :],
                                     inter3[i][:], perf_mode=DR,
                                     start=(i == IC // 4),
                                     stop=(i == IC // 2 - 1))
                of = ofp.tile([128, TOK], f32, tag="of")
                nc.vector.scalar_tensor_tensor(of[:], ops[:], 1.0 / 65536.0,
                                               of_half[oc][:], op0=ALU.mult,
                                               op1=ALU.add)
                nc.sync.dma_start(outT[128 * oc:128 * (oc + 1), :], of[:])

            p4s.close()
            ress.close()

    nc.compile()
    return nc


_NC_CACHE = {}


def kernel(**inputs):
    x = np.asarray(inputs["x"], np.float32)
    norm_w = np.asarray(inputs["norm_w"], np.float32)
    norm_b = np.asarray(inputs["norm_b"], np.float32)
    qkvw = np.asarray(inputs["attn_qkvw"], np.float32)
    qkvb = np.asarray(inputs["attn_qkvb"], np.float32)
    attn_ow = np.asarray(inputs["attn_ow"], np.float32)
    attn_ob = np.asarray(inputs["attn_ob"], np.float32)
    attn_nw = np.asarray(inputs["attn_nw"], np.float32)
    attn_nb = np.asarray(inputs["attn_nb"], np.float32)
    inter_w = np.asarray(inputs["inter_w"], np.float32)
    inter_b = np.asarray(inputs["inter_b"], np.float32)
    inter_w1 = np.asarray(inputs["inter_w1"], np.float32)
    output_w = np.asarray(inputs["output_w"], np.float32)
    output_b = np.asarray(inputs["output_b"], np.float32)

    X = x.reshape(T, D)
    XT = np.ascontiguousarray(X.T)          # [D, T]

    # ---- LN folds (host) ----
    wqkv_f = norm_w[:, None] * qkvw          # [D, 3D]
    bqkv_f = qkvb + norm_b @ qkvw

    w1_f = attn_nw[:, None] * inter_w        # [D, I]
    b1_f = inter_b + attn_nb @ inter_w

    assert not np.any(bqkv_f) and not np.any(attn_ob) and not np.any(b1_f) \
        and not np.any(output_b), "nonzero biases not wired in this build"

    wq_s = 32.0 * wqkv_f
    wqblk = _f8(wq_s.reshape(DC, 128, 3, DC, 128).transpose(2, 3, 1, 0, 4))
    # v weights in [d, feat] orientation, DoubleRow chunk-pairs
    wv_s = wq_s[:, 2 * D:3 * D]              # [D, D]
    wvr = _f8(wv_s.reshape(DC // 2, 2, 128, D).transpose(0, 2, 1, 3))
    owblk = _f8((32.0 * attn_ow).reshape(DC, 128, DC, 128).transpose(2, 1, 0, 3))
    w1_s = 32.0 * w1_f
    w1blk = _f8(w1_s.reshape(DC, 128, IC, 128).transpose(2, 1, 0, 3))
    w2blk = _f8((32.0 * inter_w1).reshape(DC, 128, IC, 128).transpose(2, 1, 0, 3))
    outwblk = _f8((32.0 * output_w).reshape(IC, 128, DC, 128)
                  .transpose(2, 1, 0, 3))

    if "nc" not in _NC_CACHE:
        _NC_CACHE["nc"] = _build()
    nc = _NC_CACHE["nc"]

    in_maps = []
    for c in range(NC):
        tsl = slice(TOK * c, TOK * (c + 1))
        xs = np.ascontiguousarray(XT[:, tsl])
        in_maps.append({
            "xbf": _bf16(xs),
            "wqblk": wqblk,
            "wvr": wvr,
            "owblk": owblk,
            "w1blk": w1blk,
            "w2blk": w2blk,
            "outwblk": outwblk,
        })

    global _LAST_IN_MAPS
    _LAST_IN_MAPS = in_maps
    res = run_bass_kernel_spmd(nc, in_maps, list(range(NC)))
    outT = np.concatenate([res.results[c]["outT"] for c in range(NC)], axis=1)
    return np.ascontiguousarray(outT.T).reshape(B, S, D).astype(np.float32)


if __name__ == "__main__":
    pass
